# revision 13
# baseline (speedup 1.0000x reference)
"""BiLSTM-CRF Trainium2 kernel. Self-contained.

Distribution: data-parallel over batch, B=64 -> 8 cores x 8 seqs.
Per core the device computes: input projections (fp32r GEMMs), all four
LSTM scans (L0/L1 x fwd/bwd, sequential over T with fp32r matmuls +
ACT/DVE pointwise), in-SBUF sequence reversal (lengths baked as access
patterns), and the two emission head GEMMs. Host does the embedding row
gather (pure data movement), CRF Viterbi decode (exact fp32, tiny), and
shard assembly.
"""
import sys, os
sys.path.insert(0, "/opt/trn_rl_repo")
import numpy as np

import concourse.bass as bass
import concourse.mybir as mybir
import concourse.tile as tile
from concourse.masks import make_identity
from concourse.bass_utils import run_bass_kernel_spmd

F32 = mybir.dt.float32
F32R = mybir.dt.float32r

B, T, V, D, HD, H = 64, 256, 50000, 300, 1024, 512
L0, L1 = 13, 25
BL = 8
TOK = BL * T          # 2048
G4 = 4 * H
KC0 = 3               # ceil(300/128)
NCH = TOK // 128      # 16 m-chunks


def _gate_perm():
    p = np.zeros(G4, np.int64)
    base = {0: 0, 1: H, 2: 3 * H, 3: 2 * H}   # blocks [i, f, o, g]
    for k in range(4):
        for blk in range(4):
            for u in range(128):
                p[k * 512 + blk * 128 + u] = base[blk] + k * 128 + u
    return p


GPERM = _gate_perm()


def _hoist_waits(nc):
    cnt = [0]
    for bbname, bb in nc.bb_map.items():
        insts = bb.bb.instructions
        i = 0
        while i < len(insts):
            inst = insts[i]
            si = getattr(inst, "sync_info", None)
            if si is None or not si.on_wait:
                i += 1
                continue
            tname = type(inst).__name__
            keep = 2 if tname == "InstEventSemaphore" else (
                0 if "DMA" in tname else 1)
            waits = list(si.on_wait)
            if len(waits) <= keep:
                i += 1
                continue
            hoist = waits[:-keep] if keep else waits
            remain = waits[-keep:] if keep else []
            new = []
            for w in hoist:
                cnt[0] += 1
                es = mybir.InstEventSemaphore(
                    name=f"WLG-{cnt[0]}",
                    sync_info=mybir.SyncInfo(on_wait=[w], on_update=[]))
                es.engine = inst.engine
                new.append(es)
            inst.sync_info = mybir.SyncInfo(on_wait=remain,
                                            on_update=list(si.on_update))
            insts[i:i] = new
            i += 1 + len(new)
    return cnt[0]


def _build_program(lens):
    """lens: [8] per-seq valid lengths for THIS core-group layout; the same
    program runs SPMD on all 8 cores, so lens must be identical across
    cores -> we bake the per-b reversal APs from each core's OWN lens...
    SPMD needs one program: bake using per-core different data is not
    possible, so reversal APs use full T and host pre/post handles tails.

    Trick used instead: reversal on device uses full-length (T) reversed
    copies; the host supplies xr already reverse_padded, and for the h
    reversals the device reverses the FULL axis; tails differ from the
    reference only at padded positions, which never affect valid outputs.
    """
    nc = bass.Bass()
    x_d = nc.declare_dram_parameter("x", [TOK, D], F32, isOutput=False)
    xr_d = nc.declare_dram_parameter("xr", [TOK, D], F32, isOutput=False)
    w0_d = nc.declare_dram_parameter("w0", [KC0 * 128, G4], F32, isOutput=False)
    w0b_d = nc.declare_dram_parameter("w0bw", [KC0 * 128, G4], F32, isOutput=False)
    b0f_d = nc.declare_dram_parameter("b0f", [G4], F32, isOutput=False)
    b0b_d = nc.declare_dram_parameter("b0bw", [G4], F32, isOutput=False)
    wh0f_d = nc.declare_dram_parameter("wh0f", [512, G4], F32, isOutput=False)
    wh0b_d = nc.declare_dram_parameter("wh0b", [512, G4], F32, isOutput=False)
    w1f_d = nc.declare_dram_parameter("w1f", [1024, G4], F32, isOutput=False)
    w1b_d = nc.declare_dram_parameter("w1b", [1024, G4], F32, isOutput=False)
    b1f_d = nc.declare_dram_parameter("b1f", [G4], F32, isOutput=False)
    b1b_d = nc.declare_dram_parameter("b1bw", [G4], F32, isOutput=False)
    wh1f_d = nc.declare_dram_parameter("wh1f", [512, G4], F32, isOutput=False)
    wh1b_d = nc.declare_dram_parameter("wh1b", [512, G4], F32, isOutput=False)
    hw_d = nc.declare_dram_parameter("hw", [1024, 64], F32, isOutput=False)
    hb_d = nc.declare_dram_parameter("hb", [64], F32, isOutput=False)
    rmask_d = nc.declare_dram_parameter("rmask", [TOK], F32, isOutput=False)
    pmr_d = nc.declare_dram_parameter("pmr", [TOK], F32, isOutput=False)

    gx_d = {}
    for nm in ("gx0f", "gx0b", "gx1b", "gx1f"):
        gx_d[nm] = nc.dram_tensor(nm, [TOK, G4], F32)
    em_d = nc.declare_dram_parameter("em", [TOK, 64], F32, isOutput=True)

    def lhsT_of(ft, revft):
        def f(kc, m):
            b, th = m // 2, m % 2
            src = ft if kc < 4 else revft
            kk = kc % 4
            return bass.AP(src.tensor, src.offset + kk * T * BL + (th * 128) * BL + b,
                           [[src.ap[0][0], 128], [BL, 128]])
        return f

    with tile.TileContext(nc) as tc:
        with tc.tile_pool(name="pc", bufs=1) as pc:
            ident = pc.tile([128, 128], F32, tag="ident")
            make_identity(nc, ident[:])
            i8 = pc.tile([BL, BL], F32, tag="i8")
            make_identity(nc, i8[:])
            i8r = pc.tile([BL, BL], F32R, tag="i8r")
            nc.vector.tensor_copy(i8r[:], i8[:])

            # ---------- phase 1: gx0 for both directions ----------
            with tc.tile_pool(name="p1", bufs=1) as p1, \
                 tc.tile_pool(name="ps1", bufs=2, space="PSUM") as ps1:
                w0 = _round_chunks(nc, p1, "w0", KC0, w0_d)
                w0bw = _round_chunks(nc, p1, "w0bw", KC0, w0b_d)
                brep0f = _brep(nc, p1, b0f_d, "b0f")
                brep0b = _brep(nc, p1, b0b_d, "b0b")
                pm_sb = p1.tile([128, NCH], F32, tag="pm")
                nc.sync.dma_start(out=pm_sb[:],
                                  in_=pmr_d.ap().rearrange("(c p) -> p c", p=128))
                for (xd, gxd, br, pm, ww) in ((x_d, "gx0f", brep0f, None, w0),
                                              (xr_d, "gx0b", brep0b, pm_sb, w0bw)):
                    with tc.tile_pool(name="pe" + gxd, bufs=1) as pe:
                        xT = _xT(nc, pe, p1, ps1, xd, ident, gxd)
                        _gemm(nc, p1, ps1, lambda kc, m, xT=xT:
                              xT[:, kc, m * 128:(m + 1) * 128],
                              ww, KC0, gx_d[gxd], br, pm=pm)

            # ---------- phase 2: L0 scans + gx1 ----------
            with tc.tile_pool(name="p2", bufs=1) as p2, \
                 tc.tile_pool(name="psG", bufs=1, space="PSUM") as psG, \
                 tc.tile_pool(name="psT", bufs=2, space="PSUM") as psT:
                ftf = _scan(nc, tc, p2, psG, psT, gx_d["gx0f"], wh0f_d, "f0", i8, i8r)
                ftb = _scan(nc, tc, p2, psG, psT, gx_d["gx0b"], wh0b_d, "r0", i8, i8r)
                revf = _revfull(nc, p2, ftf, "revf")
                revb = _revfull(nc, p2, ftb, "revb")
                with tc.tile_pool(name="pw1", bufs=1) as pw1, \
                     tc.tile_pool(name="psw1", bufs=2, space="PSUM") as psw1:
                    brep1f = _brep(nc, pw1, b1f_d, "b1f")
                    brep1b = _brep(nc, pw1, b1b_d, "b1b")
                    pm2 = pw1.tile([128, NCH], F32, tag="pm2")
                    nc.sync.dma_start(out=pm2[:],
                                      in_=pmr_d.ap().rearrange("(c p) -> p c", p=128))
                    for half in range(2):
                        w1h = _round_chunks(nc, pw1, "w1h", 8, w1f_d,
                                            ncol=1024, col0=half * 1024)
                        _gemm(nc, pw1, psw1, lhsT_of(ftf[:], revb[:]), w1h, 8,
                              gx_d["gx1f"], brep1f, ncol=1024, col0=half * 1024)
                        w1h = _round_chunks(nc, pw1, "w1h", 8, w1b_d,
                                            ncol=1024, col0=half * 1024)
                        _gemm(nc, pw1, psw1, lhsT_of(revf[:], ftb[:]), w1h, 8,
                              gx_d["gx1b"], brep1b, pm=pm2,
                              ncol=1024, col0=half * 1024)

            # ---------- phase 3: L1 scans + heads ----------
            with tc.tile_pool(name="p3", bufs=1) as p3, \
                 tc.tile_pool(name="psG3", bufs=1, space="PSUM") as psG3, \
                 tc.tile_pool(name="psT3", bufs=2, space="PSUM") as psT3:
                ft1f = _scan(nc, tc, p3, psG3, psT3, gx_d["gx1f"], wh1f_d, "f1", i8, i8r)
                ft1b = _scan(nc, tc, p3, psG3, psT3, gx_d["gx1b"], wh1b_d, "r1", i8, i8r)
                revb1 = _revfull(nc, p3, ft1b, "revb1")
                # heads: em[tok, 64] = out1 @ hw + hb  (cols 0:13 em0, 16:41 em1a)
                hw = _round_chunks(nc, p3, "hw", 8, hw_d, ncol=64)
                hbrep = p3.tile([128, 64], F32, tag="hbrep")
                nc.sync.dma_start(out=hbrep[:],
                                  in_=bass.AP(hb_d.ap().tensor, 0, [[0, 128], [1, 64]]))
                rmask_sb = p3.tile([128, NCH], F32, tag="rmask")
                nc.sync.dma_start(out=rmask_sb[:],
                                  in_=rmask_d.ap().rearrange("(c p) -> p c", p=128))
                lf = lhsT_of(ft1f[:], revb1[:])
                for m in range(NCH):
                    ps = psT3.tile([128, 64], F32, tag="em_ps")
                    for kc in range(8):
                        nc.tensor.matmul(ps[:], lf(kc, m), hw[:, kc, :],
                                         start=(kc == 0), stop=(kc == 7))
                    o = p3.tile([128, 64], F32, tag="em_o", bufs=2)
                    nc.vector.tensor_add(o[:], ps[:], hbrep[:])
                    # zero padded-position emissions so host viterbi sees clean data
                    o2 = p3.tile([128, 64], F32, tag="em_o2", bufs=2)
                    nc.vector.tensor_scalar_mul(o2[:], o[:], rmask_sb[:, m:m + 1])
                    nc.sync.dma_start(out=em_d.ap()[m * 128:(m + 1) * 128, :], in_=o2[:])
    return nc


def _brep(nc, pool, bias_d, tag):
    brep = pool.tile([128, G4], F32, tag=tag)
    nc.sync.dma_start(out=brep[:], in_=bass.AP(bias_d.ap().tensor, 0, [[0, 128], [1, G4]]))
    return brep


def _round_chunks(nc, pool, tag, KC, w_dram, ncol=G4, col0=0):
    w = pool.tile([128, KC, ncol], F32R, tag=tag)
    for kc in range(KC):
        stg = pool.tile([128, ncol], F32, tag=tag + "_stg", bufs=1)
        nc.sync.dma_start(out=stg[:], in_=w_dram.ap()[kc * 128:(kc + 1) * 128,
                                                      col0:col0 + ncol])
        nc.vector.tensor_copy(w[:, kc, :], stg[:])
    return w


def _xT(nc, pool_e, pool_x, psp, x_dram, ident, tag):
    x_sb = pool_e.tile([128, NCH, D], F32, tag="x_sb")
    nc.sync.dma_start(out=x_sb[:], in_=x_dram.ap().rearrange("(c p) d -> p c d", p=128))
    xT = pool_e.tile([128, KC0, TOK], F32R, tag="xT")
    for pb in (32, 64, 96):
        nc.vector.memset(xT[:].bitcast(F32)[pb:pb + 32, 2, :], 0.0)
    for c in range(NCH):
        for kc in range(KC0):
            lo = kc * 128
            hi = min(D, lo + 128)
            pt = psp.tile([128, 128], F32, tag="xT_ps")
            nc.tensor.transpose(pt[: hi - lo, :], x_sb[:, c, lo:hi], ident[:])
            nc.vector.tensor_copy(xT[: hi - lo, kc, c * 128:(c + 1) * 128],
                                  pt[: hi - lo, :])
    return xT


def _gemm(nc, pool, psp, lhsT_chunks, w, KC, gx_dram, brep, pm=None,
          ncol=G4, col0=0):
    nnb = ncol // 512
    NEG = None
    if pm is not None:
        NEG = pool.tile([128, 128], F32, tag="negc")
        nc.vector.memset(NEG[:], -80.0)
    for m in range(NCH):
        for nb in range(nnb):
            ps = psp.tile([128, 512], F32, tag="gx_ps")
            for kc in range(KC):
                nc.tensor.matmul(ps[:], lhsT_chunks(kc, m),
                                 w[:, kc, nb * 512:(nb + 1) * 512],
                                 start=(kc == 0), stop=(kc == KC - 1))
            o = pool.tile([128, 512], F32, tag="gx_o", bufs=2)
            nc.vector.tensor_add(o[:], ps[:], brep[:, col0 + nb * 512:col0 + (nb + 1) * 512])
            c0 = col0 + nb * 512
            if pm is not None:
                # i-gate block = cols [0:128] of each 512 group
                oi = pool.tile([128, 128], F32, tag="gx_oi", bufs=2)
                nc.vector.scalar_tensor_tensor(
                    oi[:], NEG[:], pm[:, m:m + 1], o[:, 0:128],
                    op0=mybir.AluOpType.mult, op1=mybir.AluOpType.add)
                nc.sync.dma_start(out=gx_dram.ap()[m * 128:(m + 1) * 128, c0:c0 + 128],
                                  in_=oi[:])
                nc.sync.dma_start(out=gx_dram.ap()[m * 128:(m + 1) * 128, c0 + 128:c0 + 512],
                                  in_=o[:, 128:512])
            else:
                nc.sync.dma_start(out=gx_dram.ap()[m * 128:(m + 1) * 128, c0:c0 + 512],
                                  in_=o[:])


def _scan(nc, tc, pool_ft, psG, psT, gx_dram, whh_dram, tag, i8, i8r):
    FT = pool_ft.tile([128, 4, T, BL], F32R, tag=tag)
    ctx = tc.tile_pool(name=tag + "_w", bufs=1)
    pool = ctx.__enter__()
    whh = _round_chunks(nc, pool, tag + "_whh", 4, whh_dram)
    c_sb = pool.tile([BL, 512], F32, tag=tag + "_c")
    nc.vector.memset(c_sb[:], 0.0)
    sig = pool.tile([BL, 4, 384], F32, tag=tag + "_sig")
    tng = pool.tile([BL, 4, 128], F32, tag=tag + "_tng")
    tnc = pool.tile([BL, 512], F32, tag=tag + "_tnc")
    tfc = pool.tile([BL, 512], F32, tag=tag + "_tfc")
    tig = pool.tile([BL, 512], F32, tag=tag + "_tig")
    h_sb = pool.tile([BL, 512], F32, tag=tag + "_h")
    AF = mybir.ActivationFunctionType
    for t in range(T):
        gx_t = pool.tile([BL, G4], F32, tag=tag + "_gx", bufs=1)
        nc.sync.dma_start(out=gx_t[:],
                          in_=gx_dram.ap().rearrange("(b t) g -> b t g", t=T)[:, t, :])
        gx_r = pool.tile([BL, G4], F32R, tag=tag + "_gxr", bufs=1)
        nc.vector.tensor_copy(gx_r[:], gx_t[:])
        G = psG.tile([BL, G4], F32, tag="scanG")
        for nb in range(4):
            nc.tensor.matmul(G[:, nb * 512:(nb + 1) * 512], i8r[:],
                             gx_r[:, nb * 512:(nb + 1) * 512],
                             start=True, stop=(t == 0), skip_group_check=True)
        if t > 0:
            for kc in range(4):
                for nb in range(4):
                    nc.tensor.matmul(G[:, nb * 512:(nb + 1) * 512],
                                     FT[:, kc, t - 1, :],
                                     whh[:, kc, nb * 512:(nb + 1) * 512],
                                     start=False, stop=(kc == 3),
                                     skip_group_check=True)
        Gv = G[:].rearrange("b (k g) -> b k g", g=512)
        nc.scalar.activation(sig[:], Gv[:, :, 0:384], AF.Sigmoid)
        nc.scalar.activation(tng[:], Gv[:, :, 384:512], AF.Tanh)
        nc.vector.tensor_mul(tfc[:].rearrange("b (k u) -> b k u", u=128),
                             sig[:, :, 128:256],
                             c_sb[:].rearrange("b (k u) -> b k u", u=128))
        nc.vector.tensor_mul(tig[:].rearrange("b (k u) -> b k u", u=128),
                             sig[:, :, 0:128], tng[:])
        nc.vector.tensor_add(c_sb[:], tfc[:], tig[:])
        nc.scalar.activation(tnc[:], c_sb[:], AF.Tanh)
        nc.vector.tensor_mul(h_sb[:].rearrange("b (k u) -> b k u", u=128),
                             sig[:, :, 256:384],
                             tnc[:].rearrange("b (k u) -> b k u", u=128))
        pt = psT.tile([128, 4, BL], F32, tag="scanpt")
        for kc in range(4):
            nc.tensor.transpose(pt[:, kc, :], h_sb[:, kc * 128:(kc + 1) * 128], i8[:])
        nc.vector.tensor_copy(FT[:, :, t, :], pt[:])
    ctx.__exit__(None, None, None)
    return FT


def _revfull(nc, pool, ft, tag):
    """rev[p, kc, t, b] = ft[p, kc, T-1-t, b] (full-axis reversal)."""
    rv = pool.tile([128, 4, T, BL], F32R, tag=tag)
    src = ft[:]
    rev_ap = bass.AP(src.tensor, src.offset + (T - 1) * BL,
                     [[src.ap[0][0], 128], [T * BL, 4], [-BL, T], [1, BL]])
    nc.vector.tensor_copy(rv[:], rev_ap)
    return rv


# ---------------- host side ----------------

def _sig(v):
    return 1.0 / (1.0 + np.exp(-v))


def _viterbi_np(em, mask, start, end, trans):
    Bn, Tn, Ln = em.shape
    score = start[None, :] + em[:, 0]
    hist = np.zeros((Tn - 1, Bn, Ln), np.int64)
    iden = np.arange(Ln)[None, :]
    for t in range(1, Tn):
        br = score[:, :, None] + trans[None]
        nsc = br.max(axis=1) + em[:, t]
        idx = br.argmax(axis=1)
        m = mask[:, t][:, None]
        hist[t - 1] = np.where(m, idx, iden)
        score = np.where(m, nsc, score)
    last = (score + end[None, :]).argmax(axis=-1)
    tags = np.zeros((Bn, Tn), np.int64)
    tags[:, Tn - 1] = last
    cur = last
    for t in range(Tn - 2, -1, -1):
        cur = hist[t][np.arange(Bn), cur]
        tags[:, t] = cur
    return np.where(mask, tags, 0)


_CACHE = {}
_LAST_DEV_S = [None]


def kernel(**inputs):
    word_batch = np.asarray(inputs["word_batch"])
    in_dtype = word_batch.dtype
    emb = np.asarray(inputs["emb"], np.float32)
    mask = word_batch > 0
    lengths = mask.sum(axis=1).astype(np.int64)

    # host prep: gather + right-aligned time reversal (index shuffles only)
    x_full = emb[word_batch].astype(np.float32)   # [B, T, D]
    xr_full = x_full[:, ::-1, :] * mask[:, ::-1, None]
    pad_right = ~mask[:, ::-1]                    # pad rows of the reversed seqs

    GP = GPERM

    def wih_pad(w, kc):
        W = np.zeros((kc * 128, G4), np.float32)
        W[:w.shape[1]] = np.asarray(w, np.float32)[GP].T
        return W

    w0 = wih_pad(inputs["lstm0_wih"][0], KC0)
    w0bw = wih_pad(inputs["lstm0_wih"][1], KC0)
    wh0f = np.asarray(inputs["lstm0_whh"][0], np.float32)[GP].T.copy()
    wh0b = np.asarray(inputs["lstm0_whh"][1], np.float32)[GP].T.copy()
    b00 = np.asarray(inputs["lstm0_b"][0], np.float32)[GP].copy()
    b01 = np.asarray(inputs["lstm0_b"][1], np.float32)[GP].copy()
    w1f = np.asarray(inputs["lstm1_wih"][0], np.float32)[GP].T.copy()
    w1b = np.asarray(inputs["lstm1_wih"][1], np.float32)[GP].T.copy()
    b10 = np.asarray(inputs["lstm1_b"][0], np.float32)[GP].copy()
    b11 = np.asarray(inputs["lstm1_b"][1], np.float32)[GP].copy()
    wh1f = np.asarray(inputs["lstm1_whh"][0], np.float32)[GP].T.copy()
    wh1b = np.asarray(inputs["lstm1_whh"][1], np.float32)[GP].T.copy()
    h0w = np.asarray(inputs["head0_w"], np.float32)   # [13, 1024]
    h0b = np.asarray(inputs["head0_b"], np.float32)
    h1w = np.asarray(inputs["head1_w"], np.float32)   # [25, 1025]
    h1b = np.asarray(inputs["head1_b"], np.float32)

    # combined head: em cols 0:13 = em0 ; 16:41 = em1 minus tags channel
    hw = np.zeros((1024, 64), np.float32)
    hw[:, 0:L0] = h0w.T
    hw[:, 16:16 + L1] = h1w[:, :1024].T
    hb = np.zeros(64, np.float32)
    hb[0:L0] = h0b
    hb[16:16 + L1] = h1b

    key = "prog"
    if key not in _CACHE:
        _CACHE[key] = _build_program(None)
        _hoist_waits(_CACHE[key])
    nc = _CACHE[key]

    in_maps = []
    for c in range(8):
        sl = slice(c * BL, (c + 1) * BL)
        rm = mask[sl].astype(np.float32).reshape(TOK)
        in_maps.append(dict(
            x=x_full[sl].reshape(TOK, D), xr=xr_full[sl].reshape(TOK, D),
            w0=w0, w0bw=w0bw, b0f=b00, b0bw=b01, wh0f=wh0f, wh0b=wh0b,
            w1f=w1f, w1b=w1b, b1f=b10, b1bw=b11, wh1f=wh1f, wh1b=wh1b,
            hw=hw, hb=hb, rmask=rm,
            pmr=pad_right[sl].astype(np.float32).reshape(TOK),
        ))

    import time as _time
    _t0 = _time.time()
    res = run_bass_kernel_spmd(nc, in_maps, list(range(8)))
    _LAST_DEV_S[0] = _time.time() - _t0

    em = np.concatenate([r["em"].reshape(BL, T, 64) for r in res.results], axis=0)
    em0 = em[:, :, 0:L0].astype(np.float32)
    em1a = em[:, :, 16:16 + L1].astype(np.float32)

    tags0 = _viterbi_np(em0, mask, np.asarray(inputs["crf0_start"], np.float32),
                        np.asarray(inputs["crf0_end"], np.float32),
                        np.asarray(inputs["crf0_trans"], np.float32))
    em1 = em1a + tags0[:, :, None].astype(np.float32) * h1w[:, 1024][None, None, :]
    em1 = em1 * mask[:, :, None]
    tags1 = _viterbi_np(em1, mask, np.asarray(inputs["crf1_start"], np.float32),
                        np.asarray(inputs["crf1_end"], np.float32),
                        np.asarray(inputs["crf1_trans"], np.float32))

    out_dtype = np.int32 if in_dtype == np.int32 else np.int64
    tags = np.stack([tags0, tags1]).astype(out_dtype)
    return tags, lengths.astype(out_dtype)
